# revision 1
# baseline (speedup 1.0000x reference)
"""GSAT graph-attention kernel for 8 Trainium2 NeuronCores.

Math (per batch b):
  h = x @ W                                     [N, 512]
  ss[i] = h[i] . a_src_flat / H ; sd[j] = h[j] . a_dst_flat / H
  t[i,j] = (ss[i] + sd[j]) * adj[i,j] + gumbel(noise[b,i,j])
  A1 = softmax_j(t) ; A2 = softmax_j(A1)
  out[b,n] = sum_i A2[i,n] * h[i] @ W_out

Sharding: 8 cores = (batch b in 0..3) x (row-half rb in 0..1).  Rows i are
sharded; both softmaxes are along j (within-row), so each core computes its
2048 rows completely and produces a partial output  outT = W_out^T h'^T
summed over its rows; host adds the two row-half partials per batch.

Device-side structure per core:
  phase 0: hT = (x W)^T for own rows, s-vectors via folded W@a weights,
           k = h @ W_out  (W_out folded BEFORE aggregation: (A^T H)Wo = A^T(H Wo))
  main:    per 128-row block: gumbel + scores via rank-2 PE matmul, two
           softmax passes on ACT (row sums via accum_out), then 8 matmuls
           k~^T @ e2 accumulated in PSUM across all 16 blocks.
  epilog:  copy the 8 PSUM accumulators out.

Normalizations are folded: 1/rowsum1 into the second Exp's per-partition
scale, 1/rowsum2 into k (k~ = k/rs2), so unnormalized e2 feeds the matmul.
"""

import os
import sys

for _p in ("/opt/trn_rl_repo",):
    if _p not in sys.path and os.path.isdir(_p):
        sys.path.insert(0, _p)

os.environ.setdefault("MYCRO_LOCAL_CACHE", "1")

import numpy as np
import ml_dtypes

B, N, IN_F, H, OUT_F = 4, 4096, 256, 8, 64
D = H * OUT_F          # 512
RB = N // 2            # 2048 rows per core
NBLK = RB // 128       # 16 row blocks per core
EPS = 1e-9
N_CORES = 8

_cache = {}


def _build_module():
    import concourse.bacc as bacc
    import concourse.tile as tile
    from concourse import mybir

    f32 = mybir.dt.float32
    f32r = mybir.dt.float32r
    bf16 = mybir.dt.bfloat16
    AF = mybir.ActivationFunctionType
    ALU = mybir.AluOpType

    nc = bacc.Bacc("TRN2", target_bir_lowering=False)

    xT_d = nc.declare_dram_parameter("xT", [IN_F, N], f32r, isOutput=False)
    xTr_d = nc.declare_dram_parameter("xTr", [IN_F, RB], f32r, isOutput=False)
    adj_d = nc.declare_dram_parameter("adj_s", [RB, N], bf16, isOutput=False)
    nz_d = nc.declare_dram_parameter("noise_s", [RB, N], f32, isOutput=False)
    W_d = nc.declare_dram_parameter("W", [IN_F, D], f32r, isOutput=False)
    wsd_d = nc.declare_dram_parameter("wsd", [IN_F, 2], f32r, isOutput=False)
    Wo_d = nc.declare_dram_parameter("W_out", [D, OUT_F], f32r, isOutput=False)
    outT_d = nc.declare_dram_parameter("outT", [OUT_F, N], f32, isOutput=True)

    with tile.TileContext(nc) as tc:
        import contextlib

        with contextlib.ExitStack() as ctx:
            pers = ctx.enter_context(tc.tile_pool(name="pers", bufs=1))
            # persistent small tensors
            sdb = pers.tile([128, N], f32)      # s_dst broadcast down partitions
            ss_col = pers.tile([128, NBLK], f32)  # ss_col[p, b] = s_src[b*128+p]
            ktil = [pers.tile([128, OUT_F], f32r, tag=f"k{ib}", name=f"k{ib}") for ib in range(NBLK)]

            epsb = pers.tile([128, 1], f32)
            nc.vector.memset(epsb, EPS)

            # ---------------- phase 0 ----------------
            with tc.tile_pool(name="p0", bufs=1) as p0, \
                 tc.tile_pool(name="ps0", bufs=2, space="PSUM") as ps0:
                xT2 = [p0.tile([128, N], f32r, tag=f"xT{fc}", name=f"xT{fc}") for fc in range(2)]
                xTr2 = [p0.tile([128, RB], f32r, tag=f"xTr{fc}", name=f"xTr{fc}") for fc in range(2)]
                Wt = [p0.tile([128, D], f32r, tag=f"W{fc}", name=f"Wti{fc}") for fc in range(2)]
                wsdt = [p0.tile([128, 2], f32r, tag=f"wsd{fc}", name=f"wsdt{fc}") for fc in range(2)]
                Wot = [p0.tile([128, OUT_F], f32r, tag=f"Wo{dc}", name=f"Wot{dc}") for dc in range(4)]
                for fc in range(2):
                    nc.sync.dma_start(out=xT2[fc], in_=xT_d[fc * 128:(fc + 1) * 128, :])
                    nc.sync.dma_start(out=xTr2[fc], in_=xTr_d[fc * 128:(fc + 1) * 128, :])
                    nc.sync.dma_start(out=Wt[fc], in_=W_d[fc * 128:(fc + 1) * 128, :])
                    nc.sync.dma_start(out=wsdt[fc], in_=wsd_d[fc * 128:(fc + 1) * 128, :])
                for dc in range(4):
                    nc.sync.dma_start(out=Wot[dc], in_=Wo_d[dc * 128:(dc + 1) * 128, :])

                # s_dst row [1, N] then broadcast down 128 partitions
                sd_row = p0.tile([1, N], f32)
                for jc in range(8):
                    sps = ps0.tile([1, 512], f32, tag="sps")
                    for fc in range(2):
                        nc.tensor.matmul(sps, wsdt[fc][:, 1:2].bitcast(f32),
                                         xT2[fc][:, jc * 512:(jc + 1) * 512].bitcast(f32),
                                         start=(fc == 0), stop=(fc == 1))
                    nc.vector.tensor_copy(sd_row[0:1, jc * 512:(jc + 1) * 512], sps)
                sd_dram = nc.dram_tensor("sd_scratch", [1, N], f32)
                nc.sync.dma_start(out=sd_dram[:], in_=sd_row)
                import concourse.bass as bass_mod
                sd_bcast = bass_mod.AP(tensor=sd_dram[:].tensor,
                                       offset=sd_dram[:].offset,
                                       ap=[[0, 128]] + list(sd_dram[:].ap)[1:])
                nc.gpsimd.dma_start(out=sdb, in_=sd_bcast)
                # ss_col[p, b] = s_src of row b*128+p (one N=1 matmul per block)
                sscol_ps = ps0.tile([128, NBLK], f32, tag="sscol")
                for ib in range(NBLK):
                    for fc in range(2):
                        nc.tensor.matmul(sscol_ps[:, ib:ib + 1],
                                         xTr2[fc][:, ib * 128:(ib + 1) * 128].bitcast(f32),
                                         wsdt[fc][:, 0:1].bitcast(f32),
                                         start=(fc == 0), stop=(fc == 1))
                nc.vector.tensor_copy(ss_col, sscol_ps)

                # hT[dc][d, i] = h[i, d] for own rows; then k = h @ W_out
                with tc.tile_pool(name="hp", bufs=1) as hp:
                    hT = [hp.tile([128, RB], f32r, tag=f"hT{dc}", name=f"hT{dc}") for dc in range(4)]
                    for dc in range(4):
                        for ic in range(RB // 512):
                            hps = ps0.tile([128, 512], f32, tag="hps")
                            for fc in range(2):
                                nc.tensor.matmul(
                                    hps,
                                    Wt[fc][:, dc * 128:(dc + 1) * 128],
                                    xTr2[fc][:, ic * 512:(ic + 1) * 512],
                                    start=(fc == 0), stop=(fc == 1))
                            nc.vector.tensor_copy(hT[dc][:, ic * 512:(ic + 1) * 512], hps)
                    for ib in range(NBLK):
                        kps = ps0.tile([128, OUT_F], f32, tag="kps")
                        for dc in range(4):
                            nc.tensor.matmul(kps,
                                             hT[dc][:, ib * 128:(ib + 1) * 128].bitcast(f32),
                                             Wot[dc].bitcast(f32),
                                             start=(dc == 0), stop=(dc == 3))
                        nc.vector.tensor_copy(ktil[ib], kps)

            # ---------------- main loop ----------------
            with tc.tile_pool(name="agg", bufs=1, space="PSUM") as aggpool, \
                 tc.tile_pool(name="stream", bufs=3) as spool, \
                 tc.tile_pool(name="smalls", bufs=4) as rpool:
                aggp = [aggpool.tile([64, 512], f32, tag=f"agg{j}", name=f"agg{j}") for j in range(8)]

                # stage A: DMA noise/adj + gumbel Ln passes + score stt,
                # emitted one block AHEAD of stage B so ACT never stalls on DVE.
                def stage_a(ib):
                    nz = spool.tile([128, N], f32, tag="nz", name=f"nz{ib}")
                    nc.sync.dma_start(out=nz, in_=nz_d[ib * 128:(ib + 1) * 128, :])
                    ad = spool.tile([128, N], bf16, tag="ad", name=f"ad{ib}")
                    nc.sync.dma_start(out=ad, in_=adj_d[ib * 128:(ib + 1) * 128, :])
                    m = spool.tile([128, N], f32r, tag="m", name=f"m{ib}")
                    # v = log(noise + EPS); g = log(EPS - v)   (in-place)
                    nc.scalar.activation(out=nz, in_=nz, func=AF.Ln, bias=epsb, scale=1.0)
                    nc.scalar.activation(out=nz, in_=nz, func=AF.Ln, bias=epsb, scale=-1.0)
                    # m = (sd[j] + ss[i]) * adj   (one fused stt)
                    nc.vector.scalar_tensor_tensor(out=m, in0=sdb,
                                                   scalar=ss_col[:, ib:ib + 1],
                                                   in1=ad, op0=ALU.add, op1=ALU.mult)
                    return nz, m

                staged = {0: stage_a(0)}
                for ib in range(NBLK):
                    if ib + 1 < NBLK:
                        staged[ib + 1] = stage_a(ib + 1)
                    nz, m = staged.pop(ib)
                    # t = m - g  (computed as (g * -1) + m)
                    nc.vector.scalar_tensor_tensor(out=m, in0=nz, scalar=-1.0, in1=m,
                                                   op0=ALU.mult, op1=ALU.add)

                    # e1 = exp(t), rs1 = rowsum(e1)
                    rs1 = rpool.tile([128, 1], f32, tag="rs1")
                    nc.scalar.activation(out=m, in_=m, func=AF.Exp, accum_out=rs1)
                    rs1r = rpool.tile([128, 1], f32, tag="rs1r")
                    nc.vector.reciprocal(rs1r, rs1)

                    # e2 = exp(e1/rs1), rs2 = rowsum(e2)
                    rs2 = rpool.tile([128, 1], f32, tag="rs2")
                    nc.scalar.activation(out=m, in_=m, func=AF.Exp, scale=rs1r,
                                         accum_out=rs2)
                    rs2r = rpool.tile([128, 1], f32, tag="rs2r")
                    nc.vector.reciprocal(rs2r, rs2)

                    # k~ = k / rs2
                    kt = rpool.tile([128, OUT_F], f32r, tag="kt")
                    nc.vector.tensor_scalar(out=kt, in0=ktil[ib], scalar1=rs2r,
                                            scalar2=None, op0=ALU.mult)

                    # outT += k~^T @ e2 : accumulate in PSUM across all blocks
                    for ns in range(8):
                        nc.tensor.matmul(aggp[ns], kt,
                                         m[:, ns * 512:(ns + 1) * 512],
                                         start=(ib == 0), stop=(ib == NBLK - 1))

                # ---------------- epilogue ----------------
                with tc.tile_pool(name="fin", bufs=1) as fpool:
                    outT = fpool.tile([OUT_F, N], f32)
                    for ns in range(8):
                        nc.vector.tensor_copy(
                            outT[:, ns * 512:(ns + 1) * 512], aggp[ns])
                    nc.sync.dma_start(out=outT_d[:], in_=outT)

    nc.compile()
    return nc


def _get_module():
    if "nc" not in _cache:
        _cache["nc"] = _build_module()
    return _cache["nc"]


def kernel(x, adj, noise, W, a_src, a_dst, W_out):
    from concourse.bass_utils import run_bass_kernel_spmd

    nc = _get_module()

    x = np.asarray(x, dtype=np.float32)
    adj = np.asarray(adj, dtype=np.float32)
    noise = np.asarray(noise, dtype=np.float32)
    W = np.asarray(W, dtype=np.float32)
    a_src = np.asarray(a_src, dtype=np.float32)
    a_dst = np.asarray(a_dst, dtype=np.float32)
    W_out = np.asarray(W_out, dtype=np.float32)

    # fold the per-head score weights: s = (x @ W) @ a_flat / H == x @ (W @ a_flat / H)
    w_src = (W @ a_src.reshape(-1)) / H
    w_dst = (W @ a_dst.reshape(-1)) / H
    wsd = np.ascontiguousarray(np.stack([w_src, w_dst], axis=1), dtype=np.float32)
    adj_bf = adj.astype(ml_dtypes.bfloat16)  # exact for 0/1 values
    Wc = np.ascontiguousarray(W)
    Woc = np.ascontiguousarray(W_out)

    in_maps = []
    for core in range(N_CORES):
        b, rb = core // 2, core % 2
        rows = slice(rb * RB, (rb + 1) * RB)
        xTb = np.ascontiguousarray(x[b].T)  # [IN_F, N]
        in_maps.append({
            "xT": xTb,
            "xTr": np.ascontiguousarray(xTb[:, rows]),
            "adj_s": np.ascontiguousarray(adj_bf[rows, :]),
            "noise_s": np.ascontiguousarray(noise[b, rows, :]),
            "W": Wc,
            "wsd": wsd,
            "W_out": Woc,
        })

    res = run_bass_kernel_spmd(nc, in_maps, list(range(N_CORES)))
    kernel._last_results = res

    out = np.empty((B, N, OUT_F), dtype=np.float32)
    for b in range(B):
        acc = res.results[2 * b]["outT"].astype(np.float32) + \
            res.results[2 * b + 1]["outT"].astype(np.float32)
        out[b] = acc.T
    return out



# revision 7
# speedup vs baseline: 1.3296x; 1.3296x over previous
"""GSAT graph-attention kernel for 8 Trainium2 NeuronCores.

Math (per batch b):
  h = x @ W                                     [N, 512]
  ss[i] = h[i] . w_src / H ; sd[j] = h[j] . w_dst / H
  t[i,j] = (ss[i] + sd[j]) * adj[i,j] + gumbel(noise[b,i,j])
  A1 = softmax_j(t) ; A2 = softmax_j(A1)
  out[b,n] = sum_i A2[i,n] * (h[i] @ W_out)

Key restructure vs the naive pipeline: with g = gumbel(u) = -log(v),
v = -log(u+eps), we have exp(t) = exp(m)*exp(g) = exp(m)/v, and since
adj is 0/1:  exp(m) = 1 + adj*(E_i*F_j - 1)  with E=exp(ss), F=exp(sd).
So per block:
  L  = Ln(u+eps)            (ACT; only transcendental #1)
  R  = recip_approx(L)      (DVE; R = 1/L = -1/v; sign cancels in A1)
  q  = E_i*F - 1            (DVE tensor_scalar, 4x bf16)
  w1 = q * adj              (GpSimd tensor_tensor; offload from DVE)
  e1 = (w1+1)*R, rs1=sum    (DVE affine_mul_reduce, fused accumulate)
  e2 = Exp(e1/rs1), rs2=sum (ACT; only transcendental #2)
  outT += (k/rs2)^T @ e2    (PE, bf16)
This cuts ACT from 4 full passes (2xLn+2xExp) to 2, the historic
bottleneck.  Blocks are processed in groups of 4 so the ACT engine sees
Ln,Ln,Ln,Ln,Exp,Exp,Exp,Exp and reloads its function table twice per
group instead of ~4x per block.

Sharding: 8 cores = (batch b in 0..3) x (row-half rb in 0..1); both
softmaxes are along j so each core computes its 2048 rows completely;
host adds the two row-half partial outputs per batch.

dtypes: noise stays f32 (bf16/fp16 breaks the gumbel tail for u->1);
adj/x/W/scores/exponentials all bf16 (validated: rel err ~4e-3 vs the
2e-2 gate); matmuls bf16 (4x PE rate vs f32r).
"""

import os
import sys

for _p in ("/opt/trn_rl_repo",):
    if _p not in sys.path and os.path.isdir(_p):
        sys.path.insert(0, _p)

os.environ.setdefault("MYCRO_LOCAL_CACHE", "1")

import numpy as np
import ml_dtypes

B, N, IN_F, H, OUT_F = 4, 4096, 256, 8, 64
D = H * OUT_F          # 512
RB = N // 2            # 2048 rows per core
NBLK = RB // 128       # 16 row blocks per core
GRP = 4                # blocks per ACT-table group
EPS = 1e-9
N_CORES = 8

# blocks whose w1 = q*adj runs on GpSimd instead of DVE (DVE is the
# critical engine; GpSimd 2-input rate is ~2x worse but it idles)
GP_W1 = set(range(NBLK))

_cache = {}


def _build_module():
    import concourse.bacc as bacc
    import concourse.tile as tile
    from concourse import mybir

    f32 = mybir.dt.float32
    bf16 = mybir.dt.bfloat16
    AF = mybir.ActivationFunctionType
    ALU = mybir.AluOpType

    nc = bacc.Bacc("TRN2", target_bir_lowering=False)

    xT_d = nc.declare_dram_parameter("xT", [IN_F, N], bf16, isOutput=False)
    xTr_d = nc.declare_dram_parameter("xTr", [IN_F, RB], bf16, isOutput=False)
    adj_d = nc.declare_dram_parameter("adj_s", [RB, N], bf16, isOutput=False)
    nz_d = nc.declare_dram_parameter("noise_s", [RB, N], f32, isOutput=False)
    W_d = nc.declare_dram_parameter("W", [IN_F, D], bf16, isOutput=False)
    wsd_d = nc.declare_dram_parameter("wsd", [IN_F, 2], bf16, isOutput=False)
    Wo_d = nc.declare_dram_parameter("W_out", [D, OUT_F], bf16, isOutput=False)
    outT_d = nc.declare_dram_parameter("outT", [OUT_F, N], f32, isOutput=True)

    with tile.TileContext(nc) as tc:
        import contextlib

        with contextlib.ExitStack() as ctx:
            pers = ctx.enter_context(tc.tile_pool(name="pers", bufs=1))
            Fb = pers.tile([128, N], bf16)         # exp(sd) broadcast down parts
            E_col = pers.tile([128, NBLK], f32)    # exp(ss) per own row
            ktil = [pers.tile([128, OUT_F], bf16, tag=f"k{ib}", name=f"k{ib}")
                    for ib in range(NBLK)]
            epsb = pers.tile([128, 1], f32)
            nc.vector.memset(epsb, EPS)

            # ---------------- phase 0 ----------------
            with tc.tile_pool(name="p0", bufs=1) as p0, \
                 tc.tile_pool(name="ps0", bufs=2, space="PSUM") as ps0:
                xT2 = [p0.tile([128, N], bf16, tag=f"xT{fc}", name=f"xT{fc}") for fc in range(2)]
                xTr2 = [p0.tile([128, RB], bf16, tag=f"xTr{fc}", name=f"xTr{fc}") for fc in range(2)]
                Wt = [p0.tile([128, D], bf16, tag=f"W{fc}", name=f"Wti{fc}") for fc in range(2)]
                wsdt = [p0.tile([128, 2], bf16, tag=f"wsd{fc}", name=f"wsdt{fc}") for fc in range(2)]
                Wot = [p0.tile([128, OUT_F], bf16, tag=f"Wo{dc}", name=f"Wot{dc}") for dc in range(4)]
                for fc in range(2):
                    nc.sync.dma_start(out=xT2[fc], in_=xT_d[fc * 128:(fc + 1) * 128, :])
                    nc.sync.dma_start(out=xTr2[fc], in_=xTr_d[fc * 128:(fc + 1) * 128, :])
                    nc.sync.dma_start(out=Wt[fc], in_=W_d[fc * 128:(fc + 1) * 128, :])
                    nc.sync.dma_start(out=wsdt[fc], in_=wsd_d[fc * 128:(fc + 1) * 128, :])
                for dc in range(4):
                    nc.sync.dma_start(out=Wot[dc], in_=Wo_d[dc * 128:(dc + 1) * 128, :])

                # sd row [1, N] -> exp -> broadcast down 128 partitions
                sd_row = p0.tile([1, N], f32)
                for jc in range(8):
                    sps = ps0.tile([1, 512], f32, tag="sps")
                    for fc in range(2):
                        nc.tensor.matmul(sps, wsdt[fc][:, 1:2],
                                         xT2[fc][:, jc * 512:(jc + 1) * 512],
                                         start=(fc == 0), stop=(fc == 1))
                    nc.vector.tensor_copy(sd_row[0:1, jc * 512:(jc + 1) * 512], sps)
                F_row = p0.tile([1, N], bf16)
                nc.scalar.activation(out=F_row, in_=sd_row, func=AF.Exp)
                F_dram = nc.dram_tensor("F_scratch", [1, N], bf16)
                nc.sync.dma_start(out=F_dram[:], in_=F_row)
                import concourse.bass as bass_mod
                F_bcast = bass_mod.AP(tensor=F_dram[:].tensor,
                                      offset=F_dram[:].offset,
                                      ap=[[0, 128]] + list(F_dram[:].ap)[1:])
                nc.gpsimd.dma_start(out=Fb, in_=F_bcast)

                # ss_col[p, ib] = ss of row ib*128+p, then E = exp(ss)
                sscol_ps = ps0.tile([128, NBLK], f32, tag="sscol")
                for ib in range(NBLK):
                    for fc in range(2):
                        nc.tensor.matmul(sscol_ps[:, ib:ib + 1],
                                         xTr2[fc][:, ib * 128:(ib + 1) * 128],
                                         wsdt[fc][:, 0:1],
                                         start=(fc == 0), stop=(fc == 1))
                nc.scalar.activation(out=E_col, in_=sscol_ps, func=AF.Exp)

                # hT[dc][d, i] = h[i, d] for own rows; then k = h @ W_out
                with tc.tile_pool(name="hp", bufs=1) as hp:
                    hT = [hp.tile([128, RB], bf16, tag=f"hT{dc}", name=f"hT{dc}") for dc in range(4)]
                    for dc in range(4):
                        for ic in range(RB // 512):
                            hps = ps0.tile([128, 512], f32, tag="hps")
                            for fc in range(2):
                                nc.tensor.matmul(
                                    hps,
                                    Wt[fc][:, dc * 128:(dc + 1) * 128],
                                    xTr2[fc][:, ic * 512:(ic + 1) * 512],
                                    start=(fc == 0), stop=(fc == 1))
                            nc.vector.tensor_copy(hT[dc][:, ic * 512:(ic + 1) * 512], hps)
                    for ib in range(NBLK):
                        kps = ps0.tile([128, OUT_F], f32, tag="kps")
                        for dc in range(4):
                            nc.tensor.matmul(kps,
                                             hT[dc][:, ib * 128:(ib + 1) * 128],
                                             Wot[dc],
                                             start=(dc == 0), stop=(dc == 3))
                        nc.vector.tensor_copy(ktil[ib], kps)

            # ---------------- main loop ----------------
            with tc.tile_pool(name="agg", bufs=1, space="PSUM") as aggpool:
                aggp = [aggpool.tile([64, 512], f32, tag=f"agg{j}", name=f"agg{j}")
                        for j in range(8)]
                main_ctx = contextlib.ExitStack()
                pnz = main_ctx.enter_context(tc.tile_pool(name="pnz", bufs=2))
                pad = main_ctx.enter_context(tc.tile_pool(name="pad", bufs=2))
                pmid = main_ctx.enter_context(tc.tile_pool(name="pmid", bufs=2))
                pR = main_ctx.enter_context(tc.tile_pool(name="pR", bufs=2))
                pw1 = main_ctx.enter_context(tc.tile_pool(name="pw1", bufs=3))
                pe1 = main_ctx.enter_context(tc.tile_pool(name="pe1", bufs=GRP + 2))
                rpool = main_ctx.enter_context(tc.tile_pool(name="smalls", bufs=6))

                def stage_a(ib):
                    nz = pnz.tile([128, N], f32, tag="nz", name=f"nz{ib}")
                    nc.sync.dma_start(out=nz, in_=nz_d[ib * 128:(ib + 1) * 128, :])
                    ad = pad.tile([128, N], bf16, tag="ad", name=f"ad{ib}")
                    nc.sync.dma_start(out=ad, in_=adj_d[ib * 128:(ib + 1) * 128, :])
                    # L = ln(u + eps)  (in place, f32)
                    nc.scalar.activation(out=nz, in_=nz, func=AF.Ln,
                                         bias=epsb, scale=1.0)
                    # R = 1/L (fast approx; R<0, sign cancels in A1)
                    Rb = pR.tile([128, N], f32, tag="R", name=f"R{ib}")
                    nc.vector.reciprocal_approx_fast(Rb, nz)
                    # q = E_i*F_j - 1
                    q = pmid.tile([128, N], bf16, tag="q", name=f"q{ib}")
                    nc.vector.tensor_scalar(out=q, in0=Fb,
                                            scalar1=E_col[:, ib:ib + 1],
                                            scalar2=-1.0,
                                            op0=ALU.mult, op1=ALU.add)
                    # w1 = q * adj
                    w1 = pw1.tile([128, N], bf16, tag="w1", name=f"w1{ib}")
                    eng = nc.gpsimd if ib in GP_W1 else nc.vector
                    eng.tensor_tensor(out=w1, in0=q, in1=ad, op=ALU.mult)
                    # e1 = (w1 + 1) * R ; rs1 = rowsum(e1)
                    e1 = pe1.tile([128, N], bf16, tag="e1", name=f"e1{ib}")
                    rs1 = rpool.tile([128, 1], f32, tag="rs1")
                    nc.vector.affine_mul_reduce(out=e1, accum_out=rs1,
                                                in0=w1, in1=Rb,
                                                scale=1.0, bias=1.0)
                    rs1r = rpool.tile([128, 1], f32, tag="rs1r")
                    nc.vector.reciprocal(rs1r, rs1)
                    return e1, rs1r

                def stage_b(ib, e1, rs1r):
                    # e2 = exp(e1/rs1), rs2 = rowsum(e2)
                    e2 = pmid.tile([128, N], bf16, tag="e2", name=f"e2{ib}")
                    rs2 = rpool.tile([128, 1], f32, tag="rs2")
                    nc.scalar.activation(out=e2, in_=e1, func=AF.Exp,
                                         scale=rs1r, accum_out=rs2)
                    rs2r = rpool.tile([128, 1], f32, tag="rs2r")
                    nc.vector.reciprocal(rs2r, rs2)
                    # k~ = k / rs2
                    kt = rpool.tile([128, OUT_F], bf16, tag="kt")
                    nc.vector.tensor_scalar(out=kt, in0=ktil[ib], scalar1=rs2r,
                                            scalar2=None, op0=ALU.mult)
                    # outT += k~^T @ e2 : accumulate in PSUM across blocks
                    for ns in range(8):
                        nc.tensor.matmul(aggp[ns], kt,
                                         e2[:, ns * 512:(ns + 1) * 512],
                                         start=(ib == 0), stop=(ib == NBLK - 1))

                for g in range(NBLK // GRP):
                    blocks = range(g * GRP, (g + 1) * GRP)
                    staged = [stage_a(ib) for ib in blocks]
                    for ib, (e1, rs1r) in zip(blocks, staged):
                        stage_b(ib, e1, rs1r)

                # ---------------- epilogue ----------------
                main_ctx.close()
                with tc.tile_pool(name="fin", bufs=1) as fpool:
                    outT = fpool.tile([OUT_F, N], f32)
                    for ns in range(8):
                        nc.vector.tensor_copy(
                            outT[:, ns * 512:(ns + 1) * 512], aggp[ns])
                    nc.sync.dma_start(out=outT_d[:], in_=outT)

    nc.compile()
    return nc


def _get_module():
    if "nc" not in _cache:
        _cache["nc"] = _build_module()
    return _cache["nc"]


def kernel(x, adj, noise, W, a_src, a_dst, W_out):
    from concourse.bass_utils import run_bass_kernel_spmd

    nc = _get_module()

    x = np.asarray(x, dtype=np.float32)
    adj = np.asarray(adj, dtype=np.float32)
    noise = np.asarray(noise, dtype=np.float32)
    W = np.asarray(W, dtype=np.float32)
    a_src = np.asarray(a_src, dtype=np.float32)
    a_dst = np.asarray(a_dst, dtype=np.float32)
    W_out = np.asarray(W_out, dtype=np.float32)

    # fold the per-head score weights: s = (x @ W) @ a_flat / H == x @ (W @ a_flat / H)
    w_src = (W @ a_src.reshape(-1)) / H
    w_dst = (W @ a_dst.reshape(-1)) / H
    wsd = np.ascontiguousarray(
        np.stack([w_src, w_dst], axis=1)).astype(ml_dtypes.bfloat16)
    adj_bf = adj.astype(ml_dtypes.bfloat16)  # exact for 0/1 values
    Wc = np.ascontiguousarray(W).astype(ml_dtypes.bfloat16)
    Woc = np.ascontiguousarray(W_out).astype(ml_dtypes.bfloat16)

    in_maps = []
    for core in range(N_CORES):
        b, rb = core // 2, core % 2
        rows = slice(rb * RB, (rb + 1) * RB)
        xTb = np.ascontiguousarray(x[b].T).astype(ml_dtypes.bfloat16)
        in_maps.append({
            "xT": xTb,
            "xTr": np.ascontiguousarray(xTb[:, rows]),
            "adj_s": np.ascontiguousarray(adj_bf[rows, :]),
            "noise_s": np.ascontiguousarray(noise[b, rows, :]),
            "W": Wc,
            "wsd": wsd,
            "W_out": Woc,
        })

    res = run_bass_kernel_spmd(nc, in_maps, list(range(N_CORES)))
    kernel._last_results = res

    out = np.empty((B, N, OUT_F), dtype=np.float32)
    for b in range(B):
        acc = res.results[2 * b]["outT"].astype(np.float32) + \
            res.results[2 * b + 1]["outT"].astype(np.float32)
        out[b] = acc.T
    return out
